# revision 1
# baseline (speedup 1.0000x reference)
"""Trainium2 Bass kernel for nn_ExpertsFeedForward (MoE expert-choice routing).

Sharding: expert-parallel with host-side token dispatch. The router
(softmax + expert-choice top-k) runs on host, as does the gather/scatter-add
"all-to-all" and all rank-1 epilogues (bv/sbv biases, jump expert, score
scaling). Each core runs two dense FFN blocks over pre-gathered tokens:

  block A (CAP=1638 tok): cores 0-6 -> that core's FF expert; core 7 -> shared
  block B (1844 tok): a slice of the shared expert's tokens

so per-core compute is a balanced 3482 token-passes of gelu-FFN. All device
DMAs are large contiguous transfers of host-prelaid tensors (no gpsimd
gather/scatter, no transposing DMA), gelu runs on the Activation engine with
the bk bias fused, GEMM2 runs transposed (output [d, tok], exact chunk widths,
no 128-row tile rounding; host detransposes), and careful DMA issue order
(x chunk -> wk quarters -> wv on one queue, next block's head pre-issued
during the previous block's tail) keeps PE busy ~98% of the span.
"""

import numpy as np
import ml_dtypes

import concourse.bass as bass
import concourse.mybir as mybir
import concourse.bacc as bacc
import concourse.tile as tile
from concourse.bass_utils import run_bass_kernel_spmd

BF16 = mybir.dt.bfloat16
F32 = mybir.dt.float32

NC = 8            # cores
B, S = 8, 2048
D = 1024          # d_model
H = 4096          # d_ff
FF = 7            # matmul experts
E = 8             # router experts (7 FF + jump)
NTOK = 16384      # total tokens
CAP = 1638        # expert capacity
KD = D // 128     # 8 contraction tiles over d
MH = H // 128     # 32 tiles over h
MH2 = MH // 2     # 16 (half-split for DMA descriptor sizing)
MH4 = MH // 4     # 8  (quarter-split so GEMM1 starts after the first piece)
ND2 = 2           # 512-wide n chunks over D
NTA = 1664        # block-A layout stride (13*128 >= CAP; only CAP computed)
NTB = 1844        # block-B (shared) tokens per core (balanced: CAP+NTB==3482)
NT = NTA + NTB    # 3508 token slots per core
GELU = mybir.ActivationFunctionType.Gelu_apprx_tanh


CHW = 256  # token chunk width (also GEMM2 psum width; 1KB psum tiles)


def _chunks(base, total):
    """Split [base, base+total) into CHW-token chunks (+mult-of-128 tail)."""
    out = []
    t = 0
    while t < total:
        cw = min(CHW, total - t)
        out.append((base + t, cw))
        t += cw
    return out


def build_program(reps=1):
    nc = bacc.Bacc("TRN2", target_bir_lowering=False, debug=False, num_devices=NC)

    # ---- per-core external inputs (host-prelaid, all contiguous) ----
    xT = nc.dram_tensor("xT", [128, KD, NT], BF16, kind="ExternalInput")
    wks = nc.dram_tensor("wks", [2, 4, 128, KD * MH4 * 128], BF16,
                         kind="ExternalInput")
    wvs = nc.dram_tensor("wvs", [2, 2, 128, MH2 * KD * 128], BF16,
                         kind="ExternalInput")
    bks = nc.dram_tensor("bks", [2, 128, MH], F32, kind="ExternalInput")
    out = nc.dram_tensor("out", [128, KD, NT], BF16, kind="ExternalOutput")

    with tile.TileContext(nc) as tc:
        with (
            tc.tile_pool(name="w", bufs=1) as wp,
            tc.tile_pool(name="io", bufs=2) as iop,
            tc.tile_pool(name="act", bufs=2) as ap_,
            tc.tile_pool(name="o", bufs=3) as op_,
            tc.tile_pool(name="ps1", bufs=2, space="PSUM") as pp1,
            tc.tile_pool(name="ps2", bufs=4, space="PSUM") as pp2,
        ):
            def issue_xc(t0, cw):
                xc = iop.tile([128, KD, CHW], BF16, tag="xc")
                nc.sync.dma_start(out=xc[:, :, :cw], in_=xT[:, :, t0:t0 + cw])
                return xc

            def issue_head(blk):
                """x chunk 0, then wk (quarter 0 first so GEMM1 can start as
                soon as it lands), then bias, then the remaining quarters."""
                t0, cw = _chunks(NTA, NTB)[0] if blk else _chunks(0, CAP)[0]
                xc0 = issue_xc(t0, cw)
                wk_sb = wp.tile([128, 4, KD * MH4 * 128], BF16, tag="wk")
                half = KD * MH4 * 128 // 2  # m-major: first piece covers mq 0-3
                nc.sync.dma_start(out=wk_sb[:, 0, :half], in_=wks[blk, 0, :, :half])
                bk_sb = iop.tile([128, MH], F32, tag="bk")
                nc.sync.dma_start(out=bk_sb[:], in_=bks[blk])
                nc.sync.dma_start(out=wk_sb[:, 0, half:], in_=wks[blk, 0, :, half:])
                for q in range(1, 4):
                    nc.sync.dma_start(out=wk_sb[:, q], in_=wks[blk, q])
                return xc0, wk_sb, bk_sb

            for _rep in range(reps):
                heads = {0: issue_head(0)}
                for blk in range(2):
                    # block A computes only the CAP real tokens (tail chunk 102)
                    chunks = _chunks(0 if blk == 0 else NTA,
                                     CAP if blk == 0 else NTB)
                    xc0, wk_sb, bk_sb = heads.pop(blk)
                    xcs = {0: xc0, 1: issue_xc(*chunks[1])}
                    # wv after the x/wk head so it can't jump the DMA queue
                    wv_sb = wp.tile([128, 2, MH2 * KD * 128], BF16, tag="wv")
                    for h in range(2):
                        nc.sync.dma_start(out=wv_sb[:, h], in_=wvs[blk, h])

                    for ci, (t0, cw) in enumerate(chunks):
                        xc = xcs.pop(ci, None)
                        if xc is None:
                            xc = issue_xc(t0, cw)
                        hT = ap_.tile([128, MH, CHW], BF16, tag="hT")
                        for m in range(MH):
                            ps1 = pp1.tile([128, CHW], F32, tag="ps1")
                            for k in range(KD):
                                st = ((m % MH4) * KD + k) * 128
                                nc.tensor.matmul(
                                    ps1[:, :cw], wk_sb[:, m // MH4, st:st + 128],
                                    xc[:, k, :cw], start=(k == 0), stop=(k == KD - 1))
                            nc.scalar.activation(hT[:, m, :cw], ps1[:, :cw], GELU,
                                                 bias=bk_sb[:, m:m + 1])
                        if blk == 0 and ci == len(chunks) - 1:
                            # next block's head streams in during this block's
                            # tail GEMM2 (wk WAR clears at the G1 just issued)
                            heads[1] = issue_head(1)
                        oT = op_.tile([128, KD, CHW], BF16, tag="o")
                        for nd in range(KD):
                            ps2 = pp2.tile([128, CHW], F32, tag="ps2")
                            for m in range(MH):
                                off = ((m % MH2) * KD + nd) * 128
                                nc.tensor.matmul(
                                    ps2[:, :cw], wv_sb[:, m // MH2, off:off + 128],
                                    hT[:, m, :cw], start=(m == 0), stop=(m == MH - 1))
                            nc.vector.tensor_copy(oT[:, nd, :cw], ps2[:, :cw])
                        nc.sync.dma_start(out=out[:, :, t0:t0 + cw], in_=oT[:, :, :cw])

    nc.compile()
    return nc


def _bf(a):
    return np.ascontiguousarray(a.astype(ml_dtypes.bfloat16))


def host_route(x_flat, gate_W, gate_b, temperature):
    """Replicates the reference router + expert-choice top-k in numpy."""
    logits = x_flat.astype(np.float32) @ gate_W + gate_b
    t = max(float(np.asarray(temperature).reshape(-1)[0]), 0.1)
    z = logits / t
    z = z - z.max(axis=1, keepdims=True)
    p = np.exp(z)
    p = p / p.sum(axis=1, keepdims=True)
    order = np.argsort(-p, axis=0, kind="stable")
    sel = order[:CAP]  # [CAP, 8]
    return p, sel


def _wk_layout(Wk):
    """[D, H] f32 -> [4, 128, MH4*KD*128] bf16, quarter-major over H with
    m-major interior so the first half-quarter already covers m-tiles 0-3."""
    quarters = []
    for q in range(4):
        w = Wk[:, q * (H // 4):(q + 1) * (H // 4)]
        quarters.append(w.reshape(KD, 128, MH4, 128).transpose(1, 2, 0, 3)
                        .reshape(128, MH4 * KD * 128))
    return _bf(np.stack(quarters))


def _wv_layout(Wv):
    """[H, D] f32 -> [2, 128, MH2*KD*128] bf16 (half-major over H; interior
    m-major then d-tile so GEMM2's lhsT is a flat 128-col slice)."""
    halves = []
    for h in range(2):
        w = Wv[h * (H // 2):(h + 1) * (H // 2), :]
        halves.append(w.reshape(MH2, 128, KD, 128).transpose(1, 0, 2, 3)
                      .reshape(128, MH2 * KD * 128))
    return _bf(np.stack(halves))


def _shared_slices(c):
    """Token ranges of the flat [16384] space assigned to core c's B block
    (core 7 also covers [b0, b0+CAP) in its A block)."""
    if c < 7:
        return (c * NTB, (c + 1) * NTB)
    return (7 * NTB, NTOK)  # core 7: first CAP in block A, rest in block B


def prepare_in_maps(inputs):
    x = np.asarray(inputs["x"], np.float32).reshape(NTOK, D)
    p, sel = host_route(
        x, np.asarray(inputs["gate_W"], np.float32),
        np.asarray(inputs["gate_b"], np.float32),
        np.asarray(inputs["temperature"], np.float32),
    )

    sWk = np.asarray(inputs["sWk"], np.float32)
    sWv = np.asarray(inputs["sWv"], np.float32)
    sbk = np.asarray(inputs["sbk"], np.float32)
    Wk = np.asarray(inputs["Wk"], np.float32)
    Wv = np.asarray(inputs["Wv"], np.float32)
    bk = np.asarray(inputs["bk"], np.float32)

    swk_l = _wk_layout(sWk)
    swv_l = _wv_layout(sWv)
    sbk_l = np.ascontiguousarray(sbk.reshape(MH, 128).T)

    in_maps = []
    for c in range(NC):
        xtok = np.zeros((NT, D), np.float32)
        if c < 7:
            g = np.sort(sel[:, c])
            xtok[:CAP] = x[g]
            b0, b1 = _shared_slices(c)
            xtok[NTA:NTA + (b1 - b0)] = x[b0:b1]
            wks_c = np.stack([_wk_layout(Wk[c]), swk_l])
            wvs_c = np.stack([_wv_layout(Wv[c]), swv_l])
            bks_c = np.stack([np.ascontiguousarray(bk[c].reshape(MH, 128).T), sbk_l])
        else:
            b0, b1 = _shared_slices(c)  # 12908..16384
            xtok[:CAP] = x[b0:b0 + CAP]
            xtok[NTA:NTA + (b1 - b0 - CAP)] = x[b0 + CAP:b1]
            wks_c = np.stack([swk_l, swk_l])
            wvs_c = np.stack([swv_l, swv_l])
            bks_c = np.stack([sbk_l, sbk_l])
        xT_c = np.ascontiguousarray(
            xtok.T.reshape(KD, 128, NT).transpose(1, 0, 2))
        in_maps.append({
            "xT": _bf(xT_c),
            "wks": wks_c, "wvs": wvs_c,
            "bks": np.ascontiguousarray(bks_c),
        })
    return in_maps, p, sel


_CACHED = None


def kernel(**inputs):
    global _CACHED
    if _CACHED is None:
        _CACHED = build_program()
    nc = _CACHED
    in_maps, p, sel = prepare_in_maps(inputs)
    res = run_bass_kernel_spmd(nc, in_maps, list(range(NC)))
    outs = [np.asarray(res.results[c]["out"], ml_dtypes.bfloat16)
            .astype(np.float32).transpose(2, 1, 0).reshape(NT, D)
            for c in range(NC)]

    bv = np.asarray(inputs["bv"], np.float32)
    sbv = np.asarray(inputs["sbv"], np.float32)
    jump = np.asarray(inputs["jump"], np.float32)

    final = np.empty((NTOK, D), np.float32)
    # shared expert (+ sbv) for every token, from the owning core
    for c in range(7):
        b0, b1 = _shared_slices(c)
        final[b0:b1] = outs[c][NTA:NTA + (b1 - b0)]
    b0, b1 = _shared_slices(7)
    final[b0:b0 + CAP] = outs[7][:CAP]
    final[b0 + CAP:b1] = outs[7][NTA:NTA + (b1 - b0 - CAP)]
    final += sbv
    # FF experts: score-scaled, bv folded, scatter-added to owning tokens
    for c in range(7):
        g = np.sort(sel[:, c])
        final[g] += (outs[c][:CAP] + bv[c]) * p[g, c][:, None]
    # constant 'jump' expert
    m7 = sel[:, FF]
    final[m7] += jump[None, :] * p[m7, FF][:, None]
    return final.reshape(B, S, D)


if __name__ == "__main__":
    d = np.load("/root/problem/ref_inputs.npz")
    exp = np.load("/root/problem/ref_out.npy")
    got = kernel(**{k: d[k] for k in d.files})
    err = np.abs(got - exp)
    print("absmax rel:", err.max() / np.abs(exp).max())
    print("rms rel:", np.sqrt((err ** 2).mean()) / exp.std())



# revision 2
# speedup vs baseline: 1.1711x; 1.1711x over previous
"""Trainium2 Bass kernel for nn_ExpertsFeedForward (MoE expert-choice routing).

Sharding: expert-parallel with host-side token dispatch (router softmax +
expert-choice top-k, gather/scatter "all-to-all", rank-1 epilogues all on
host). Each core runs two dense gelu-FFN blocks over pre-gathered tokens:

  block A (CAP=1638 tok): cores 0-6 -> that core's FF expert; core 7 -> shared
  block B (1844 tok): a slice of the shared expert's tokens

Compute path: fp8(e4m3) hi+lo split matmuls in DoubleRow perf mode. Every
operand X is represented as X_hi + X_lo (fp8 quantize, then fp8 quantize the
residual at the same scale); each GEMM runs 3 DoubleRow passes
(hi*hi + lo*hi + hi*lo, the lo*lo term is negligible) contracting 2 k-tiles
per instruction at 0.5 PE cycles/row -- 3/4 the PE time of bf16 at bf16-level
accuracy (~3.4e-3 vs the 2e-2 gate). Weights are pre-scaled by 512 on host
(fp8 normal range), un-scaled via the activation's scale and a host-side
divide. h = gelu(z) is split on-device: scalar engine emits h_hi (fp8) and
h_bf (bf16), vector engine emits h_lo = h_bf - h_hi. x/out use padded
chunk-major DRAM layouts so every DMA moves >=512B contiguous rows.
"""

import numpy as np
import ml_dtypes

import concourse.bass as bass
import concourse.mybir as mybir
import concourse.bacc as bacc
import concourse.tile as tile
from concourse.bass_utils import run_bass_kernel_spmd

F8 = mybir.dt.float8e4
BF16 = mybir.dt.bfloat16
F32 = mybir.dt.float32
DR = mybir.MatmulPerfMode.DoubleRow
GELU = mybir.ActivationFunctionType.Gelu_apprx_tanh

NC = 8            # cores
B, S = 8, 2048
D = 1024          # d_model
H = 4096          # d_ff
FF = 7            # matmul experts
E = 8             # router experts (7 FF + jump)
NTOK = 16384      # total tokens
CAP = 1638        # expert capacity
KD = D // 128     # 8 contraction tiles over d
KD2 = KD // 2     # 4 k-pairs (DoubleRow contracts 2 tiles/inst)
MH = H // 128     # 32 tiles over h
MH2 = MH // 2     # 16 h k-pairs for GEMM2
ND = D // 128     # 8 output d tiles
NTB = 1844        # block-B (shared) tokens per core (balanced: CAP+NTB==3482)
CHW = 256         # token chunk width
NCH_A = 7         # A chunks: 6*256 + 102
NCH_B = 8         # B chunks: 7*256 + 52
NCH = NCH_A + NCH_B
BB0 = NCH_A * CHW  # padded row where block B starts (1792)
SW = 512.0        # weight pre-scale (fp8 dynamic range)

A_CHUNKS = [(ci, CHW if ci < NCH_A - 1 else CAP - (NCH_A - 1) * CHW)
            for ci in range(NCH_A)]
B_CHUNKS = [(NCH_A + ci, CHW if ci < NCH_B - 1 else NTB - (NCH_B - 1) * CHW)
            for ci in range(NCH_B)]

WKROW = MH * 2 * KD2 * 2 * 128   # 65536 fp8 bytes per partition
WVROW = ND * 2 * MH2 * 2 * 128   # 65536


def build_program():
    nc = bacc.Bacc("TRN2", target_bir_lowering=False, debug=False, num_devices=NC)

    xh = nc.dram_tensor("xh", [128, NCH, KD, CHW], F8, kind="ExternalInput")
    xl = nc.dram_tensor("xl", [128, NCH, KD, CHW], F8, kind="ExternalInput")
    wks = nc.dram_tensor("wks", [2, 128, WKROW], F8, kind="ExternalInput")
    wvs = nc.dram_tensor("wvs", [2, 128, WVROW], F8, kind="ExternalInput")
    bks = nc.dram_tensor("bks", [2, 128, MH], F32, kind="ExternalInput")
    out = nc.dram_tensor("out", [128, NCH, KD, CHW], BF16, kind="ExternalOutput")

    with tile.TileContext(nc) as tc:
        with (
            tc.tile_pool(name="w", bufs=1) as wp,
            tc.tile_pool(name="io", bufs=2) as iop,
            tc.tile_pool(name="h8", bufs=2) as hp,
            tc.tile_pool(name="hb", bufs=3) as hbp,
            tc.tile_pool(name="o", bufs=2) as op_,
            tc.tile_pool(name="ps1", bufs=3, space="PSUM") as pp1,
            tc.tile_pool(name="ps2", bufs=4, space="PSUM") as pp2,
        ):
            def issue_x(ci):
                xch = iop.tile([128, KD, CHW], F8, tag="xh")
                xcl = iop.tile([128, KD, CHW], F8, tag="xl")
                nc.sync.dma_start(out=xch[:], in_=xh[:, ci])
                nc.sync.dma_start(out=xcl[:], in_=xl[:, ci])
                return xch, xcl

            # wk m-major pieces: G1 m=0 can start once the first lands
            WK_PIECES = [(0, 4), (4, 12), (12, 22), (22, 32)]
            MROW = 2 * KD2 * 2 * 128  # 2048 bytes per m-tile

            def issue_head(blk, ci0):
                xc0 = issue_x(ci0)
                wk_sb = wp.tile([128, MH, 2, KD2, 2, 128], F8, tag="wk")
                m0, m1 = WK_PIECES[0]
                nc.sync.dma_start(out=wk_sb[:, m0:m1],
                                  in_=wks[blk, :, m0 * MROW:m1 * MROW])
                bk_sb = iop.tile([128, MH], F32, tag="bk")
                nc.sync.dma_start(out=bk_sb[:], in_=bks[blk])
                for m0, m1 in WK_PIECES[1:]:
                    nc.sync.dma_start(out=wk_sb[:, m0:m1],
                                      in_=wks[blk, :, m0 * MROW:m1 * MROW])
                return xc0, wk_sb, bk_sb

            NDROW = 2 * MH2 * 2 * 128  # 8192 bytes per nd-tile
            heads = {0: issue_head(0, 0)}
            for blk in range(2):
                chunks = A_CHUNKS if blk == 0 else B_CHUNKS
                xc0, wk_sb, bk_sb = heads.pop(blk)
                xcs = {0: xc0, 1: issue_x(chunks[1][0])}
                wv_sb = wp.tile([128, ND, 2, MH2, 2, 128], F8, tag="wv")
                for n0, n1 in [(0, 3), (3, 6), (6, 8)]:
                    nc.sync.dma_start(out=wv_sb[:, n0:n1],
                                      in_=wvs[blk, :, n0 * NDROW:n1 * NDROW])

                for li, (ci, cw) in enumerate(chunks):
                    xc = xcs.pop(li, None)
                    if xc is None:
                        xc = issue_x(ci)
                    xch, xcl = xc
                    h_hi = hp.tile([128, MH, CHW], F8, tag="hh")
                    h_lo = hp.tile([128, MH, CHW], F8, tag="hl")
                    hbt = None
                    for m in range(MH):
                        ps1 = pp1.tile([128, CHW], F32, tag="ps1")
                        for j in range(KD2):
                            wh = wk_sb[:, m, 0, j]
                            wl = wk_sb[:, m, 1, j]
                            rh = xch[:, 2 * j:2 * j + 2, :cw]
                            rl = xcl[:, 2 * j:2 * j + 2, :cw]
                            nc.tensor.matmul(ps1[:, :cw], wh, rh,
                                             start=(j == 0), stop=False,
                                             perf_mode=DR)
                            nc.tensor.matmul(ps1[:, :cw], wh, rl,
                                             start=False, stop=False,
                                             perf_mode=DR)
                            nc.tensor.matmul(ps1[:, :cw], wl, rh,
                                             start=False, stop=(j == KD2 - 1),
                                             perf_mode=DR)
                        if m % 8 == 0:
                            hbt = hbp.tile([128, 8, CHW], BF16, tag="hb")
                        nc.scalar.activation(h_hi[:, m, :cw], ps1[:, :cw], GELU,
                                             bias=bk_sb[:, m:m + 1], scale=1.0 / SW)
                        nc.scalar.activation(hbt[:, m % 8, :cw], ps1[:, :cw], GELU,
                                             bias=bk_sb[:, m:m + 1], scale=1.0 / SW)
                        if m % 8 == 7:
                            nc.vector.tensor_sub(h_lo[:, m - 7:m + 1, :cw],
                                                 hbt[:, :, :cw],
                                                 h_hi[:, m - 7:m + 1, :cw])
                    if blk == 0 and li == len(chunks) - 1:
                        # next block's head streams in during this block's
                        # tail GEMM2 (wk WAR clears at the G1 just issued)
                        heads[1] = issue_head(1, B_CHUNKS[0][0])
                    oT = op_.tile([128, KD, CHW], BF16, tag="o")
                    for nd in range(ND):
                        ps2 = pp2.tile([128, CHW], F32, tag="ps2")
                        for j in range(MH2):
                            vh = wv_sb[:, nd, 0, j]
                            vl = wv_sb[:, nd, 1, j]
                            rh = h_hi[:, 2 * j:2 * j + 2, :cw]
                            rl = h_lo[:, 2 * j:2 * j + 2, :cw]
                            nc.tensor.matmul(ps2[:, :cw], vh, rh,
                                             start=(j == 0), stop=False,
                                             perf_mode=DR)
                            nc.tensor.matmul(ps2[:, :cw], vh, rl,
                                             start=False, stop=False,
                                             perf_mode=DR)
                            nc.tensor.matmul(ps2[:, :cw], vl, rh,
                                             start=False, stop=(j == MH2 - 1),
                                             perf_mode=DR)
                        nc.vector.tensor_copy(oT[:, nd, :cw], ps2[:, :cw])
                    nc.sync.dma_start(out=out[:, ci], in_=oT[:])

    nc.compile()
    return nc


def _q8(a):
    return np.asarray(a, np.float32).astype(ml_dtypes.float8_e4m3)


def _split8(a):
    hi = _q8(a)
    lo = _q8(np.asarray(a, np.float32) - hi.astype(np.float32))
    return hi, lo


def host_route(x_flat, gate_W, gate_b, temperature):
    """Replicates the reference router + expert-choice top-k in numpy."""
    logits = x_flat.astype(np.float32) @ gate_W + gate_b
    t = max(float(np.asarray(temperature).reshape(-1)[0]), 0.1)
    z = logits / t
    z = z - z.max(axis=1, keepdims=True)
    p = np.exp(z)
    p = p / p.sum(axis=1, keepdims=True)
    order = np.argsort(-p, axis=0, kind="stable")
    sel = order[:CAP]  # [CAP, 8]
    return p, sel


def _wk_layout(Wk):
    """[D, H] f32 -> [128, WKROW] fp8 pair: layout (p, m, a, j, i, c) with
    element = Q(SW*Wk)[(2j+i)*128+p, m*128+c], a = hi/lo."""
    hi, lo = _split8(SW * Wk)
    parts = []
    for q in (hi, lo):
        parts.append(q.reshape(KD2, 2, 128, MH, 128).transpose(2, 3, 0, 1, 4))
    # parts[a] is [128, MH, KD2, 2, 128]; interleave a per m
    st = np.stack(parts, axis=2)  # [128, MH, 2, KD2, 2, 128]
    return np.ascontiguousarray(st.reshape(128, WKROW))


def _wv_layout(Wv):
    """[H, D] f32 -> [128, WVROW] fp8 pair: layout (p, nd, a, j, i, c) with
    element = Q(SW*Wv)[(2j+i)*128+p, nd*128+c]."""
    hi, lo = _split8(SW * Wv)
    parts = []
    for q in (hi, lo):
        parts.append(q.reshape(MH2, 2, 128, ND, 128).transpose(2, 3, 0, 1, 4))
    st = np.stack(parts, axis=2)  # [128, ND, 2, MH2, 2, 128]
    return np.ascontiguousarray(st.reshape(128, WVROW))


def _x_layout(xtok):
    """[NCH*CHW, D] f32 (padded tokens) -> two [128, NCH, KD, CHW] fp8."""
    hi, lo = _split8(xtok)
    outs = []
    for q in (hi, lo):
        outs.append(np.ascontiguousarray(
            q.reshape(NCH, CHW, KD, 128).transpose(3, 0, 2, 1)))
    return outs


def _shared_slices(c):
    if c < 7:
        return (c * NTB, (c + 1) * NTB)
    return (7 * NTB, NTOK)  # core 7: first CAP in block A, rest in block B


def prepare_in_maps(inputs):
    x = np.asarray(inputs["x"], np.float32).reshape(NTOK, D)
    p, sel = host_route(
        x, np.asarray(inputs["gate_W"], np.float32),
        np.asarray(inputs["gate_b"], np.float32),
        np.asarray(inputs["temperature"], np.float32),
    )

    sWk = np.asarray(inputs["sWk"], np.float32)
    sWv = np.asarray(inputs["sWv"], np.float32)
    sbk = np.asarray(inputs["sbk"], np.float32)
    Wk = np.asarray(inputs["Wk"], np.float32)
    Wv = np.asarray(inputs["Wv"], np.float32)
    bk = np.asarray(inputs["bk"], np.float32)

    swk_l = _wk_layout(sWk)
    swv_l = _wv_layout(sWv)
    sbk_l = np.ascontiguousarray(sbk.reshape(MH, 128).T)

    in_maps = []
    for c in range(NC):
        xtok = np.zeros((NCH * CHW, D), np.float32)
        if c < 7:
            g = np.sort(sel[:, c])
            xtok[:CAP] = x[g]
            b0, b1 = _shared_slices(c)
            xtok[BB0:BB0 + (b1 - b0)] = x[b0:b1]
            wks_c = np.stack([_wk_layout(Wk[c]), swk_l])
            wvs_c = np.stack([_wv_layout(Wv[c]), swv_l])
            bks_c = np.stack([np.ascontiguousarray(bk[c].reshape(MH, 128).T),
                              sbk_l])
        else:
            b0, b1 = _shared_slices(c)  # 12908..16384
            xtok[:CAP] = x[b0:b0 + CAP]
            xtok[BB0:BB0 + (b1 - b0 - CAP)] = x[b0 + CAP:b1]
            wks_c = np.stack([swk_l, swk_l])
            wvs_c = np.stack([swv_l, swv_l])
            bks_c = np.stack([sbk_l, sbk_l])
        xh_c, xl_c = _x_layout(xtok)
        in_maps.append({
            "xh": xh_c, "xl": xl_c,
            "wks": np.ascontiguousarray(wks_c),
            "wvs": np.ascontiguousarray(wvs_c),
            "bks": np.ascontiguousarray(bks_c),
        })
    return in_maps, p, sel


_CACHED = None


def kernel(**inputs):
    global _CACHED
    if _CACHED is None:
        _CACHED = build_program()
    nc = _CACHED
    in_maps, p, sel = prepare_in_maps(inputs)
    res = run_bass_kernel_spmd(nc, in_maps, list(range(NC)))
    outs = [np.asarray(res.results[c]["out"], ml_dtypes.bfloat16)
            .astype(np.float32).transpose(1, 3, 2, 0).reshape(NCH * CHW, D) / SW
            for c in range(NC)]

    bv = np.asarray(inputs["bv"], np.float32)
    sbv = np.asarray(inputs["sbv"], np.float32)
    jump = np.asarray(inputs["jump"], np.float32)

    final = np.empty((NTOK, D), np.float32)
    # shared expert (+ sbv) for every token, from the owning core
    for c in range(7):
        b0, b1 = _shared_slices(c)
        final[b0:b1] = outs[c][BB0:BB0 + (b1 - b0)]
    b0, b1 = _shared_slices(7)
    final[b0:b0 + CAP] = outs[7][:CAP]
    final[b0 + CAP:b1] = outs[7][BB0:BB0 + (b1 - b0 - CAP)]
    final += sbv
    # FF experts: score-scaled, bv folded, scatter-added to owning tokens
    for c in range(7):
        g = np.sort(sel[:, c])
        final[g] += (outs[c][:CAP] + bv[c]) * p[g, c][:, None]
    # constant 'jump' expert
    m7 = sel[:, FF]
    final[m7] += jump[None, :] * p[m7, FF][:, None]
    return final.reshape(B, S, D)


if __name__ == "__main__":
    d = np.load("/root/problem/ref_inputs.npz")
    exp = np.load("/root/problem/ref_out.npy")
    got = kernel(**{k: d[k] for k in d.files})
    err = np.abs(got - exp)
    print("absmax rel:", err.max() / np.abs(exp).max())
    print("rms rel:", np.sqrt((err ** 2).mean()) / exp.std())


# revision 4
# speedup vs baseline: 1.3261x; 1.1323x over previous
"""Trainium2 Bass kernel for nn_ExpertsFeedForward (MoE expert-choice routing).

Sharding: expert-parallel with host-side token dispatch (router softmax +
expert-choice top-k, gather/scatter "all-to-all", rank-1 epilogues all on
host). Each core runs two dense gelu-FFN blocks over pre-gathered tokens:

  block A (CAP=1638 tok): cores 0-6 -> that core's FF expert; core 7 -> shared
  block B (1844 tok): a slice of the shared expert's tokens

Compute path: fp8(e4m3) hi+lo split matmuls in DoubleRow perf mode. Every
operand X is represented as X_hi + X_lo (fp8 quantize, then fp8 quantize the
residual at the same scale); each GEMM runs 3 DoubleRow passes
(hi*hi + lo*hi + hi*lo, the lo*lo term is negligible) contracting 2 k-tiles
per instruction at 0.5 PE cycles/row -- 3/4 the PE time of bf16 at bf16-level
accuracy (~3.4e-3 vs the 2e-2 gate). Weights are pre-scaled by 512 on host
(fp8 normal range), un-scaled via the activation's scale and a host-side
divide. The scalar engine emits gelu once per m-tile (bf16); the vector
engine derives h_hi (fp8 cast) and h_lo (residual sub) per 4-m-tile group so
the scalar engine stays under PE's GEMM1 pace. GEMM2 runs its h_lo pass last
so the residual is never on the critical path. Weights load as per-piece
tiles (own WAR scopes) so the next block's stream starts as soon as the
previous block's first m-tiles retire; x/out use padded chunk-major DRAM
layouts so every DMA moves >=512B contiguous rows.
"""

import numpy as np
import ml_dtypes

import concourse.bass as bass
import concourse.mybir as mybir
import concourse.bacc as bacc
import concourse.tile as tile
from concourse.bass_utils import run_bass_kernel_spmd

F8 = mybir.dt.float8e4
BF16 = mybir.dt.bfloat16
F32 = mybir.dt.float32
DR = mybir.MatmulPerfMode.DoubleRow
GELU = mybir.ActivationFunctionType.Gelu_apprx_tanh

NC = 8            # cores
B, S = 8, 2048
D = 1024          # d_model
H = 4096          # d_ff
FF = 7            # matmul experts
E = 8             # router experts (7 FF + jump)
NTOK = 16384      # total tokens
CAP = 1638        # expert capacity
KD = D // 128     # 8 contraction tiles over d
KD2 = KD // 2     # 4 k-pairs (DoubleRow contracts 2 tiles/inst)
MH = H // 128     # 32 tiles over h
MH2 = MH // 2     # 16 h k-pairs for GEMM2
ND = D // 128     # 8 output d tiles
NTB = 1844        # block-B (shared) tokens per core (balanced: CAP+NTB==3482)
CHW = 256         # token chunk width
NCH_A = 7         # A chunks: 6*256 + 102
NCH_B = 8         # B chunks: 7*256 + 52
NCH = NCH_A + NCH_B
BB0 = NCH_A * CHW  # padded row where block B starts (1792)
SW = 512.0        # weight pre-scale (fp8 dynamic range)

# near-uniform chunk widths (first A chunk full-width to cover the initial
# weight stream; no narrow tail chunk whose per-m act overhead would outrun
# PE): each chunk ci owns padded token rows [ci*CHW, ci*CHW + w)
A_WIDTHS = [256, 231, 231, 230, 230, 230, 230]            # sum = CAP
B_WIDTHS = [256, 227, 227, 227, 227, 227, 227, 226]       # sum = NTB
assert sum(A_WIDTHS) == CAP and sum(B_WIDTHS) == NTB
A_CHUNKS = [(ci, w) for ci, w in enumerate(A_WIDTHS)]
B_CHUNKS = [(NCH_A + ci, w) for ci, w in enumerate(B_WIDTHS)]
A_ROWS = np.concatenate([np.arange(ci * CHW, ci * CHW + w)
                         for ci, w in A_CHUNKS])
B_ROWS = np.concatenate([np.arange(ci * CHW, ci * CHW + w)
                         for ci, w in B_CHUNKS])

WKROW = MH * 2 * KD2 * 2 * 128   # 65536 fp8 bytes per partition
WVROW = ND * 2 * MH2 * 2 * 128   # 65536

# weight stream pieces: each is its own tile (own WAR scope) so the next
# block's DMA starts as soon as this block's reads of that piece retire,
# and fine granularity lets GEMM1 ride the (serialized) DMA stream
WK_PIECES = [(m, m + 2) for m in range(0, MH, 2)]     # m-tile ranges
WV_PIECES = [(0, 2), (2, 4), (4, 6), (6, 8)]          # nd-tile ranges
MROW = 2 * KD2 * 2 * 128   # 2048 B per m-tile
NDROW = 2 * MH2 * 2 * 128  # 8192 B per nd-tile
MG = 4                     # m-tiles per h-residual group


def build_program():
    nc = bacc.Bacc("TRN2", target_bir_lowering=False, debug=False, num_devices=NC)

    xh = nc.dram_tensor("xh", [128, NCH, KD, CHW], F8, kind="ExternalInput")
    xl = nc.dram_tensor("xl", [128, NCH, KD, CHW], F8, kind="ExternalInput")
    wks = nc.dram_tensor("wks", [2, 128, WKROW], F8, kind="ExternalInput")
    wvs = nc.dram_tensor("wvs", [2, 128, WVROW], F8, kind="ExternalInput")
    bks = nc.dram_tensor("bks", [2, 128, MH], F32, kind="ExternalInput")
    out = nc.dram_tensor("out", [128, NCH, KD, CHW], BF16, kind="ExternalOutput")

    with tile.TileContext(nc) as tc:
        with (
            tc.tile_pool(name="w", bufs=1) as wp,
            tc.tile_pool(name="io", bufs=2) as iop,
            tc.tile_pool(name="h8", bufs=2) as hp,
            tc.tile_pool(name="hb", bufs=3) as hbp,
            tc.tile_pool(name="o", bufs=2) as op_,
            tc.tile_pool(name="ps1", bufs=3, space="PSUM") as pp1,
            tc.tile_pool(name="ps2", bufs=4, space="PSUM") as pp2,
        ):
            def issue_x(ci):
                xch = iop.tile([128, KD, CHW], F8, tag="xh")
                xcl = iop.tile([128, KD, CHW], F8, tag="xl")
                nc.sync.dma_start(out=xch[:], in_=xh[:, ci])
                nc.sync.dma_start(out=xcl[:], in_=xl[:, ci])
                return xch, xcl

            def issue_head(blk, ci0):
                xc0 = issue_x(ci0)
                wkp = []
                for pi, (m0, m1) in enumerate(WK_PIECES):
                    t = wp.tile([128, m1 - m0, 2, KD2, 2, 128], F8,
                                tag=f"wk{pi}")
                    nc.sync.dma_start(out=t[:],
                                      in_=wks[blk, :, m0 * MROW:m1 * MROW])
                    wkp.append(t)
                    if pi == 0:
                        bk_sb = iop.tile([128, MH], F32, tag="bk")
                        nc.sync.dma_start(out=bk_sb[:], in_=bks[blk])
                return xc0, wkp, bk_sb

            def wk_at(wkp, m):
                for pi, (m0, m1) in enumerate(WK_PIECES):
                    if m0 <= m < m1:
                        return wkp[pi], m - m0
                raise AssertionError

            def wv_at(wvp, nd):
                for pi, (n0, n1) in enumerate(WV_PIECES):
                    if n0 <= nd < n1:
                        return wvp[pi], nd - n0
                raise AssertionError

            heads = {0: issue_head(0, 0)}
            for blk in range(2):
                chunks = A_CHUNKS if blk == 0 else B_CHUNKS
                xc0, wkp, bk_sb = heads.pop(blk)
                xcs = {0: xc0, 1: issue_x(chunks[1][0])}
                wvp = []
                for pi, (n0, n1) in enumerate(WV_PIECES):
                    t = wp.tile([128, n1 - n0, 2, MH2, 2, 128], F8,
                                tag=f"wv{pi}")
                    nc.sync.dma_start(out=t[:],
                                      in_=wvs[blk, :, n0 * NDROW:n1 * NDROW])
                    wvp.append(t)

                for li, (ci, cw) in enumerate(chunks):
                    xc = xcs.pop(li, None)
                    if xc is None:
                        xc = issue_x(ci)
                    xch, xcl = xc
                    h_hi = hp.tile([128, MH, CHW], F8, tag="hh")
                    h_lo = hp.tile([128, MH, CHW], F8, tag="hl")
                    hbt = None
                    for m in range(MH):
                        wkt, ml = wk_at(wkp, m)
                        ps1 = pp1.tile([128, CHW], F32, tag="ps1")
                        for j in range(KD2):
                            wh = wkt[:, ml, 0, j]
                            wl = wkt[:, ml, 1, j]
                            rh = xch[:, 2 * j:2 * j + 2, :cw]
                            rl = xcl[:, 2 * j:2 * j + 2, :cw]
                            nc.tensor.matmul(ps1[:, :cw], wh, rh,
                                             start=(j == 0), stop=False,
                                             perf_mode=DR)
                            nc.tensor.matmul(ps1[:, :cw], wh, rl,
                                             start=False, stop=False,
                                             perf_mode=DR)
                            nc.tensor.matmul(ps1[:, :cw], wl, rh,
                                             start=False, stop=(j == KD2 - 1),
                                             perf_mode=DR)
                        if m % MG == 0:
                            hbt = hbp.tile([128, MG, CHW], BF16, tag="hb")
                        nc.scalar.activation(hbt[:, m % MG, :cw], ps1[:, :cw],
                                             GELU, bias=bk_sb[:, m:m + 1],
                                             scale=1.0 / SW)
                        if m % MG == MG - 1:
                            g0 = m - MG + 1
                            nc.vector.tensor_copy(h_hi[:, g0:m + 1, :cw],
                                                  hbt[:, :, :cw])
                            nc.vector.tensor_sub(h_lo[:, g0:m + 1, :cw],
                                                 hbt[:, :, :cw],
                                                 h_hi[:, g0:m + 1, :cw])
                    if blk == 0 and li == len(chunks) - 1:
                        # next block's head streams in during this block's
                        # tail GEMM2 (wk piece WARs clear per-piece in the G1
                        # just issued)
                        heads[1] = issue_head(1, B_CHUNKS[0][0])
                    oT = op_.tile([128, KD, CHW], BF16, tag="o")
                    for nd in range(ND):
                        wvt, nl = wv_at(wvp, nd)
                        ps2 = pp2.tile([128, CHW], F32, tag="ps2")
                        # pass-major: both h_hi passes first, h_lo pass last
                        # (the residual is produced late by DVE)
                        for j in range(MH2):
                            nc.tensor.matmul(
                                ps2[:, :cw], wvt[:, nl, 0, j],
                                h_hi[:, 2 * j:2 * j + 2, :cw],
                                start=(j == 0), stop=False, perf_mode=DR)
                        for j in range(MH2):
                            nc.tensor.matmul(
                                ps2[:, :cw], wvt[:, nl, 1, j],
                                h_hi[:, 2 * j:2 * j + 2, :cw],
                                start=False, stop=False, perf_mode=DR)
                        for j in range(MH2):
                            nc.tensor.matmul(
                                ps2[:, :cw], wvt[:, nl, 0, j],
                                h_lo[:, 2 * j:2 * j + 2, :cw],
                                start=False, stop=(j == MH2 - 1), perf_mode=DR)
                        nc.vector.tensor_copy(oT[:, nd, :cw], ps2[:, :cw])
                    nc.sync.dma_start(out=out[:, ci], in_=oT[:])

    nc.compile()
    return nc


def _q8(a):
    return np.asarray(a, np.float32).astype(ml_dtypes.float8_e4m3)


def _split8(a):
    hi = _q8(a)
    lo = _q8(np.asarray(a, np.float32) - hi.astype(np.float32))
    return hi, lo


def host_route(x_flat, gate_W, gate_b, temperature):
    """Replicates the reference router + expert-choice top-k in numpy."""
    logits = x_flat.astype(np.float32) @ gate_W + gate_b
    t = max(float(np.asarray(temperature).reshape(-1)[0]), 0.1)
    z = logits / t
    z = z - z.max(axis=1, keepdims=True)
    p = np.exp(z)
    p = p / p.sum(axis=1, keepdims=True)
    order = np.argsort(-p, axis=0, kind="stable")
    sel = order[:CAP]  # [CAP, 8]
    return p, sel


def _wk_layout(Wk):
    """[D, H] f32 -> [128, WKROW] fp8 pair: layout (p, m, a, j, i, c) with
    element = Q(SW*Wk)[(2j+i)*128+p, m*128+c], a = hi/lo."""
    hi, lo = _split8(SW * Wk)
    parts = [q.reshape(KD2, 2, 128, MH, 128).transpose(2, 3, 0, 1, 4)
             for q in (hi, lo)]
    st = np.stack(parts, axis=2)  # [128, MH, 2, KD2, 2, 128]
    return np.ascontiguousarray(st.reshape(128, WKROW))


def _wv_layout(Wv):
    """[H, D] f32 -> [128, WVROW] fp8 pair: layout (p, nd, a, j, i, c) with
    element = Q(SW*Wv)[(2j+i)*128+p, nd*128+c]."""
    hi, lo = _split8(SW * Wv)
    parts = [q.reshape(MH2, 2, 128, ND, 128).transpose(2, 3, 0, 1, 4)
             for q in (hi, lo)]
    st = np.stack(parts, axis=2)  # [128, ND, 2, MH2, 2, 128]
    return np.ascontiguousarray(st.reshape(128, WVROW))


def _x_layout(xtok):
    """[NCH*CHW, D] f32 (padded tokens) -> two [128, NCH, KD, CHW] fp8."""
    hi, lo = _split8(xtok)
    return [np.ascontiguousarray(
        q.reshape(NCH, CHW, KD, 128).transpose(3, 0, 2, 1)) for q in (hi, lo)]


def _shared_slices(c):
    if c < 7:
        return (c * NTB, (c + 1) * NTB)
    return (7 * NTB, NTOK)  # core 7: first CAP in block A, rest in block B


def prepare_in_maps(inputs):
    x = np.asarray(inputs["x"], np.float32).reshape(NTOK, D)
    p, sel = host_route(
        x, np.asarray(inputs["gate_W"], np.float32),
        np.asarray(inputs["gate_b"], np.float32),
        np.asarray(inputs["temperature"], np.float32),
    )

    sWk = np.asarray(inputs["sWk"], np.float32)
    sWv = np.asarray(inputs["sWv"], np.float32)
    sbk = np.asarray(inputs["sbk"], np.float32)
    Wk = np.asarray(inputs["Wk"], np.float32)
    Wv = np.asarray(inputs["Wv"], np.float32)
    bk = np.asarray(inputs["bk"], np.float32)

    swk_l = _wk_layout(sWk)
    swv_l = _wv_layout(sWv)
    sbk_l = np.ascontiguousarray(sbk.reshape(MH, 128).T)

    in_maps = []
    for c in range(NC):
        xtok = np.zeros((NCH * CHW, D), np.float32)
        if c < 7:
            g = np.sort(sel[:, c])
            xtok[:CAP] = x[g]
            b0, b1 = _shared_slices(c)
            xtok[BB0:BB0 + (b1 - b0)] = x[b0:b1]
            wks_c = np.stack([_wk_layout(Wk[c]), swk_l])
            wvs_c = np.stack([_wv_layout(Wv[c]), swv_l])
            bks_c = np.stack([np.ascontiguousarray(bk[c].reshape(MH, 128).T),
                              sbk_l])
        else:
            b0, b1 = _shared_slices(c)  # 12908..16384
            xtok[:CAP] = x[b0:b0 + CAP]
            xtok[BB0:BB0 + (b1 - b0 - CAP)] = x[b0 + CAP:b1]
            wks_c = np.stack([swk_l, swk_l])
            wvs_c = np.stack([swv_l, swv_l])
            bks_c = np.stack([sbk_l, sbk_l])
        xh_c, xl_c = _x_layout(xtok)
        in_maps.append({
            "xh": xh_c, "xl": xl_c,
            "wks": np.ascontiguousarray(wks_c),
            "wvs": np.ascontiguousarray(wvs_c),
            "bks": np.ascontiguousarray(bks_c),
        })
    return in_maps, p, sel


_CACHED = None


def kernel(**inputs):
    global _CACHED
    if _CACHED is None:
        _CACHED = build_program()
    nc = _CACHED
    in_maps, p, sel = prepare_in_maps(inputs)
    res = run_bass_kernel_spmd(nc, in_maps, list(range(NC)))
    outs = [np.asarray(res.results[c]["out"], ml_dtypes.bfloat16)
            .astype(np.float32).transpose(1, 3, 2, 0).reshape(NCH * CHW, D) / SW
            for c in range(NC)]

    bv = np.asarray(inputs["bv"], np.float32)
    sbv = np.asarray(inputs["sbv"], np.float32)
    jump = np.asarray(inputs["jump"], np.float32)

    final = np.empty((NTOK, D), np.float32)
    # shared expert (+ sbv) for every token, from the owning core
    for c in range(7):
        b0, b1 = _shared_slices(c)
        final[b0:b1] = outs[c][BB0:BB0 + (b1 - b0)]
    b0, b1 = _shared_slices(7)
    final[b0:b0 + CAP] = outs[7][:CAP]
    final[b0 + CAP:b1] = outs[7][BB0:BB0 + (b1 - b0 - CAP)]
    final += sbv
    # FF experts: score-scaled, bv folded, scatter-added to owning tokens
    for c in range(7):
        g = np.sort(sel[:, c])
        final[g] += (outs[c][:CAP] + bv[c]) * p[g, c][:, None]
    # constant 'jump' expert
    m7 = sel[:, FF]
    final[m7] += jump[None, :] * p[m7, FF][:, None]
    return final.reshape(B, S, D)


if __name__ == "__main__":
    d = np.load("/root/problem/ref_inputs.npz")
    exp = np.load("/root/problem/ref_out.npy")
    got = kernel(**{k: d[k] for k in d.files})
    err = np.abs(got - exp)
    print("absmax rel:", err.max() / np.abs(exp).max())
    print("rms rel:", np.sqrt((err ** 2).mean()) / exp.std())


# revision 13
# speedup vs baseline: 1.3502x; 1.0182x over previous
"""Trainium2 Bass kernel for nn_ExpertsFeedForward (MoE expert-choice routing).

Sharding: expert-parallel with host-side token dispatch (router softmax +
expert-choice top-k, gather/scatter "all-to-all", rank-1 epilogues all on
host). Each core runs two dense gelu-FFN blocks over pre-gathered tokens:

  block A (CAP=1638 tok): cores 0-6 -> that core's FF expert; core 7 -> shared
  block B (1844 tok): a slice of the shared expert's tokens

Compute path: fp8(e4m3) hi+lo split matmuls in DoubleRow perf mode. Every
operand X is represented as X_hi + X_lo (fp8 quantize, then fp8 quantize the
residual at the same scale); each GEMM runs 3 DoubleRow passes
(hi*hi + lo*hi + hi*lo, the lo*lo term is negligible) contracting 2 k-tiles
per instruction at 0.5 PE cycles/row -- 3/4 the PE time of bf16 at bf16-level
accuracy (~3.4e-3 vs the 2e-2 gate). Weights are pre-scaled by 512 on host
(fp8 normal range), un-scaled via the activation's scale and a host-side
divide. The scalar engine emits gelu once per m-tile (bf16); the vector
engine derives h_hi (fp8 cast) and h_lo (residual sub) per 4-m-tile group so
the scalar engine stays under PE's GEMM1 pace. GEMM2 runs its h_lo pass last
so the residual is never on the critical path. Weights load as per-piece
tiles (own WAR scopes) so the next block's stream starts as soon as the
previous block's first m-tiles retire; x/out use padded chunk-major DRAM
layouts so every DMA moves >=512B contiguous rows.
"""

import numpy as np
import ml_dtypes

import concourse.bass as bass
import concourse.mybir as mybir
import concourse.bacc as bacc
import concourse.tile as tile
from concourse.bass_utils import run_bass_kernel_spmd

F8 = mybir.dt.float8e4
BF16 = mybir.dt.bfloat16
F32 = mybir.dt.float32
DR = mybir.MatmulPerfMode.DoubleRow
GELU = mybir.ActivationFunctionType.Gelu_apprx_tanh

NC = 8            # cores
B, S = 8, 2048
D = 1024          # d_model
H = 4096          # d_ff
FF = 7            # matmul experts
E = 8             # router experts (7 FF + jump)
NTOK = 16384      # total tokens
CAP = 1638        # expert capacity
KD = D // 128     # 8 contraction tiles over d
KD2 = KD // 2     # 4 k-pairs (DoubleRow contracts 2 tiles/inst)
MH = H // 128     # 32 tiles over h
MH2 = MH // 2     # 16 h k-pairs for GEMM2
ND = D // 128     # 8 output d tiles
NTB = 1844        # block-B (shared) tokens per core (balanced: CAP+NTB==3482)
CHW = 256         # token chunk width
NCH_A = 7         # A chunks: 6*256 + 102
NCH_B = 8         # B chunks: 7*256 + 52
NCH = NCH_A + NCH_B
BB0 = NCH_A * CHW  # padded row where block B starts (1792)
SW = 512.0        # weight pre-scale (fp8 dynamic range)

# near-uniform chunk widths (first A chunk full-width to cover the initial
# weight stream; no narrow tail chunk whose per-m act overhead would outrun
# PE): each chunk ci owns padded token rows [ci*CHW, ci*CHW + w)
A_WIDTHS = [256, 231, 231, 230, 230, 230, 230]            # sum = CAP
B_WIDTHS = [256, 227, 227, 227, 227, 227, 227, 226]       # sum = NTB
assert sum(A_WIDTHS) == CAP and sum(B_WIDTHS) == NTB
A_CHUNKS = [(ci, w) for ci, w in enumerate(A_WIDTHS)]
B_CHUNKS = [(NCH_A + ci, w) for ci, w in enumerate(B_WIDTHS)]
A_ROWS = np.concatenate([np.arange(ci * CHW, ci * CHW + w)
                         for ci, w in A_CHUNKS])
B_ROWS = np.concatenate([np.arange(ci * CHW, ci * CHW + w)
                         for ci, w in B_CHUNKS])

WKROW = MH * 2 * KD2 * 2 * 128   # 65536 fp8 bytes per partition
WVROW = ND * 2 * MH2 * 2 * 128   # 65536

# weight stream pieces: each is its own tile (own WAR scope) so the next
# block's DMA starts as soon as this block's reads of that piece retire,
# and fine granularity lets GEMM1 ride the (serialized) DMA stream
WK_PIECES = [(m, m + 2) for m in range(0, MH, 2)]     # m-tile ranges
WV_PIECES = [(0, 2), (2, 4), (4, 6), (6, 8)]          # nd-tile ranges
MROW = 2 * KD2 * 2 * 128   # 2048 B per m-tile
NDROW = 2 * MH2 * 2 * 128  # 8192 B per nd-tile
MG = 4                     # m-tiles per h-residual group


def build_program():
    nc = bacc.Bacc("TRN2", target_bir_lowering=False, debug=False, num_devices=NC)

    xh = nc.dram_tensor("xh", [128, NCH, KD, CHW], F8, kind="ExternalInput")
    xl = nc.dram_tensor("xl", [128, NCH, KD, CHW], F8, kind="ExternalInput")
    wks = nc.dram_tensor("wks", [2, 128, WKROW], F8, kind="ExternalInput")
    wvs = nc.dram_tensor("wvs", [2, 128, WVROW], F8, kind="ExternalInput")
    bks = nc.dram_tensor("bks", [2, 128, MH], F32, kind="ExternalInput")
    out = nc.dram_tensor("out", [128, NCH, KD, CHW], BF16, kind="ExternalOutput")

    with tile.TileContext(nc) as tc:
        with (
            tc.tile_pool(name="w", bufs=1) as wp,
            tc.tile_pool(name="io", bufs=2) as iop,
            tc.tile_pool(name="h8", bufs=2) as hp,
            tc.tile_pool(name="hb", bufs=3) as hbp,
            tc.tile_pool(name="o", bufs=2) as op_,
            tc.tile_pool(name="ps1", bufs=3, space="PSUM") as pp1,
            tc.tile_pool(name="ps2", bufs=4, space="PSUM") as pp2,
        ):
            def issue_x(ci):
                xch = iop.tile([128, KD, CHW], F8, tag="xh")
                xcl = iop.tile([128, KD, CHW], F8, tag="xl")
                nc.sync.dma_start(out=xch[:], in_=xh[:, ci])
                nc.sync.dma_start(out=xcl[:], in_=xl[:, ci])
                return xch, xcl

            def issue_head(blk, ci0):
                wkp = [wp.tile([128, m1 - m0, 2, KD2, 2, 128], F8,
                               tag=f"wk{pi}", name=f"wk{pi}")
                       for pi, (m0, m1) in enumerate(WK_PIECES)]
                m0, m1 = WK_PIECES[0]
                nc.sync.dma_start(out=wkp[0][:],
                                  in_=wks[blk, :, m0 * MROW:m1 * MROW])
                xc0 = issue_x(ci0)
                bk_sb = iop.tile([128, MH], F32, tag="bk")
                nc.sync.dma_start(out=bk_sb[:], in_=bks[blk])
                for pi, (m0, m1) in enumerate(WK_PIECES[1:], start=1):
                    nc.sync.dma_start(out=wkp[pi][:],
                                      in_=wks[blk, :, m0 * MROW:m1 * MROW])
                return xc0, wkp, bk_sb

            def wk_at(wkp, m):
                for pi, (m0, m1) in enumerate(WK_PIECES):
                    if m0 <= m < m1:
                        return wkp[pi], m - m0
                raise AssertionError

            def wv_at(wvp, nd):
                for pi, (n0, n1) in enumerate(WV_PIECES):
                    if n0 <= nd < n1:
                        return wvp[pi], nd - n0
                raise AssertionError

            heads = {0: issue_head(0, 0)}
            for blk in range(2):
                chunks = A_CHUNKS if blk == 0 else B_CHUNKS
                xc0, wkp, bk_sb = heads.pop(blk)
                xcs = {0: xc0, 1: issue_x(chunks[1][0])}
                wvp = []
                for pi, (n0, n1) in enumerate(WV_PIECES):
                    t = wp.tile([128, n1 - n0, 2, MH2, 2, 128], F8,
                                tag=f"wv{pi}")
                    nc.sync.dma_start(out=t[:],
                                      in_=wvs[blk, :, n0 * NDROW:n1 * NDROW])
                    wvp.append(t)

                for li, (ci, cw) in enumerate(chunks):
                    xc = xcs.pop(li, None)
                    if xc is None:
                        xc = issue_x(ci)
                    if li + 1 < len(chunks) and (li + 1) not in xcs:
                        xcs[li + 1] = issue_x(chunks[li + 1][0])
                    xch, xcl = xc
                    h_hi = hp.tile([128, MH, CHW], F8, tag="hh")
                    h_lo = hp.tile([128, MH, CHW], F8, tag="hl")
                    hbt = None
                    for m in range(MH):
                        wkt, ml = wk_at(wkp, m)
                        ps1 = pp1.tile([128, CHW], F32, tag="ps1")
                        for j in range(KD2):
                            wh = wkt[:, ml, 0, j]
                            wl = wkt[:, ml, 1, j]
                            rh = xch[:, 2 * j:2 * j + 2, :cw]
                            rl = xcl[:, 2 * j:2 * j + 2, :cw]
                            nc.tensor.matmul(ps1[:, :cw], wh, rh,
                                             start=(j == 0), stop=False,
                                             perf_mode=DR)
                            nc.tensor.matmul(ps1[:, :cw], wh, rl,
                                             start=False, stop=False,
                                             perf_mode=DR)
                            nc.tensor.matmul(ps1[:, :cw], wl, rh,
                                             start=False, stop=(j == KD2 - 1),
                                             perf_mode=DR)
                        if m % MG == 0:
                            hbt = hbp.tile([128, MG, CHW], BF16, tag="hb")
                        nc.scalar.activation(hbt[:, m % MG, :cw], ps1[:, :cw],
                                             GELU, bias=bk_sb[:, m:m + 1],
                                             scale=1.0 / SW)
                        if m % MG == MG - 1:
                            g0 = m - MG + 1
                            nc.vector.tensor_copy(h_hi[:, g0:m + 1, :cw],
                                                  hbt[:, :, :cw])
                            nc.vector.tensor_sub(h_lo[:, g0:m + 1, :cw],
                                                 hbt[:, :, :cw],
                                                 h_hi[:, g0:m + 1, :cw])
                    if blk == 0 and li == len(chunks) - 1:
                        # next block's head streams in during this block's
                        # tail GEMM2 (wk piece WARs clear per-piece in the G1
                        # just issued)
                        heads[1] = issue_head(1, B_CHUNKS[0][0])
                    oT = op_.tile([128, KD, CHW], BF16, tag="o")
                    for nd in range(ND):
                        wvt, nl = wv_at(wvp, nd)
                        ps2 = pp2.tile([128, CHW], F32, tag="ps2")
                        # pass-major: both h_hi passes first, h_lo pass last
                        # (the residual is produced late by DVE)
                        for j in range(MH2):
                            nc.tensor.matmul(
                                ps2[:, :cw], wvt[:, nl, 0, j],
                                h_hi[:, 2 * j:2 * j + 2, :cw],
                                start=(j == 0), stop=False, perf_mode=DR)
                        for j in range(MH2):
                            nc.tensor.matmul(
                                ps2[:, :cw], wvt[:, nl, 1, j],
                                h_hi[:, 2 * j:2 * j + 2, :cw],
                                start=False, stop=False, perf_mode=DR)
                        for j in range(MH2):
                            nc.tensor.matmul(
                                ps2[:, :cw], wvt[:, nl, 0, j],
                                h_lo[:, 2 * j:2 * j + 2, :cw],
                                start=False, stop=(j == MH2 - 1), perf_mode=DR)
                        nc.vector.tensor_copy(oT[:, nd, :cw], ps2[:, :cw])
                        if blk == 1 and li == len(chunks) - 1 and nd == 3:
                            # final chunk: drain the first half early so the
                            # end-of-program DMA chain is half as long
                            nc.sync.dma_start(out=out[:, ci, 0:4],
                                              in_=oT[:, 0:4])
                    if blk == 1 and li == len(chunks) - 1:
                        nc.sync.dma_start(out=out[:, ci, 4:KD], in_=oT[:, 4:KD])
                    else:
                        nc.sync.dma_start(out=out[:, ci], in_=oT[:])

    nc.compile()
    return nc


def _q8(a):
    return np.asarray(a, np.float32).astype(ml_dtypes.float8_e4m3)


def _split8(a):
    hi = _q8(a)
    lo = _q8(np.asarray(a, np.float32) - hi.astype(np.float32))
    return hi, lo


def host_route(x_flat, gate_W, gate_b, temperature):
    """Replicates the reference router + expert-choice top-k in numpy."""
    logits = x_flat.astype(np.float32) @ gate_W + gate_b
    t = max(float(np.asarray(temperature).reshape(-1)[0]), 0.1)
    z = logits / t
    z = z - z.max(axis=1, keepdims=True)
    p = np.exp(z)
    p = p / p.sum(axis=1, keepdims=True)
    order = np.argsort(-p, axis=0, kind="stable")
    sel = order[:CAP]  # [CAP, 8]
    return p, sel


def _wk_layout(Wk):
    """[D, H] f32 -> [128, WKROW] fp8 pair: layout (p, m, a, j, i, c) with
    element = Q(SW*Wk)[(2j+i)*128+p, m*128+c], a = hi/lo."""
    hi, lo = _split8(SW * Wk)
    parts = [q.reshape(KD2, 2, 128, MH, 128).transpose(2, 3, 0, 1, 4)
             for q in (hi, lo)]
    st = np.stack(parts, axis=2)  # [128, MH, 2, KD2, 2, 128]
    return np.ascontiguousarray(st.reshape(128, WKROW))


def _wv_layout(Wv):
    """[H, D] f32 -> [128, WVROW] fp8 pair: layout (p, nd, a, j, i, c) with
    element = Q(SW*Wv)[(2j+i)*128+p, nd*128+c]."""
    hi, lo = _split8(SW * Wv)
    parts = [q.reshape(MH2, 2, 128, ND, 128).transpose(2, 3, 0, 1, 4)
             for q in (hi, lo)]
    st = np.stack(parts, axis=2)  # [128, ND, 2, MH2, 2, 128]
    return np.ascontiguousarray(st.reshape(128, WVROW))


def _x_layout(xtok):
    """[NCH*CHW, D] f32 (padded tokens) -> two [128, NCH, KD, CHW] fp8."""
    hi, lo = _split8(xtok)
    return [np.ascontiguousarray(
        q.reshape(NCH, CHW, KD, 128).transpose(3, 0, 2, 1)) for q in (hi, lo)]


def _shared_slices(c):
    if c < 7:
        return (c * NTB, (c + 1) * NTB)
    return (7 * NTB, NTOK)  # core 7: first CAP in block A, rest in block B


def prepare_in_maps(inputs):
    x = np.asarray(inputs["x"], np.float32).reshape(NTOK, D)
    p, sel = host_route(
        x, np.asarray(inputs["gate_W"], np.float32),
        np.asarray(inputs["gate_b"], np.float32),
        np.asarray(inputs["temperature"], np.float32),
    )

    sWk = np.asarray(inputs["sWk"], np.float32)
    sWv = np.asarray(inputs["sWv"], np.float32)
    sbk = np.asarray(inputs["sbk"], np.float32)
    Wk = np.asarray(inputs["Wk"], np.float32)
    Wv = np.asarray(inputs["Wv"], np.float32)
    bk = np.asarray(inputs["bk"], np.float32)

    swk_l = _wk_layout(sWk)
    swv_l = _wv_layout(sWv)
    sbk_l = np.ascontiguousarray(sbk.reshape(MH, 128).T)

    in_maps = []
    for c in range(NC):
        xtok = np.zeros((NCH * CHW, D), np.float32)
        if c < 7:
            g = np.sort(sel[:, c])
            xtok[A_ROWS] = x[g]
            b0, b1 = _shared_slices(c)
            xtok[B_ROWS] = x[b0:b1]
            wks_c = np.stack([_wk_layout(Wk[c]), swk_l])
            wvs_c = np.stack([_wv_layout(Wv[c]), swv_l])
            bks_c = np.stack([np.ascontiguousarray(bk[c].reshape(MH, 128).T),
                              sbk_l])
        else:
            b0, b1 = _shared_slices(c)  # 12908..16384
            xtok[A_ROWS] = x[b0:b0 + CAP]
            xtok[B_ROWS[:b1 - b0 - CAP]] = x[b0 + CAP:b1]
            wks_c = np.stack([swk_l, swk_l])
            wvs_c = np.stack([swv_l, swv_l])
            bks_c = np.stack([sbk_l, sbk_l])
        xh_c, xl_c = _x_layout(xtok)
        in_maps.append({
            "xh": xh_c, "xl": xl_c,
            "wks": np.ascontiguousarray(wks_c),
            "wvs": np.ascontiguousarray(wvs_c),
            "bks": np.ascontiguousarray(bks_c),
        })
    return in_maps, p, sel


_CACHED = None


def kernel(**inputs):
    global _CACHED
    if _CACHED is None:
        _CACHED = build_program()
    nc = _CACHED
    in_maps, p, sel = prepare_in_maps(inputs)
    res = run_bass_kernel_spmd(nc, in_maps, list(range(NC)))
    outs = [np.asarray(res.results[c]["out"], ml_dtypes.bfloat16)
            .astype(np.float32).transpose(1, 3, 2, 0).reshape(NCH * CHW, D) / SW
            for c in range(NC)]

    bv = np.asarray(inputs["bv"], np.float32)
    sbv = np.asarray(inputs["sbv"], np.float32)
    jump = np.asarray(inputs["jump"], np.float32)

    final = np.empty((NTOK, D), np.float32)
    # shared expert (+ sbv) for every token, from the owning core
    for c in range(7):
        b0, b1 = _shared_slices(c)
        final[b0:b1] = outs[c][B_ROWS]
    b0, b1 = _shared_slices(7)
    final[b0:b0 + CAP] = outs[7][A_ROWS]
    final[b0 + CAP:b1] = outs[7][B_ROWS[:b1 - b0 - CAP]]
    final += sbv
    # FF experts: score-scaled, bv folded, scatter-added to owning tokens
    for c in range(7):
        g = np.sort(sel[:, c])
        final[g] += (outs[c][A_ROWS] + bv[c]) * p[g, c][:, None]
    # constant 'jump' expert
    m7 = sel[:, FF]
    final[m7] += jump[None, :] * p[m7, FF][:, None]
    return final.reshape(B, S, D)


if __name__ == "__main__":
    d = np.load("/root/problem/ref_inputs.npz")
    exp = np.load("/root/problem/ref_out.npy")
    got = kernel(**{k: d[k] for k in d.files})
    err = np.abs(got - exp)
    print("absmax rel:", err.max() / np.abs(exp).max())
    print("rms rel:", np.sqrt((err ** 2).mean()) / exp.std())
